# revision 1
# baseline (speedup 1.0000x reference)
"""GNN neighbor-max kernel — ap_gather d=8 channel-block design.

Per core: 2 samples, batch-parallel across the 8 NeuronCores. Per sample:
  table xe[16g+q, n, j] = x[8q+j, n]  (one full copy per GPSIMD group,
  128KB/partition) so ONE gather index fetches all 128 channels of a node.
  Group g owns nodes [g*512, (g+1)*512); its index list packs 17 slots per
  node (16 neighbors + the node itself, folding the final self-max into the
  K-reduce): I_g[n_local*17 + k].
  16 chunks x 544 idx: ap_gather -> gt [128, 544*8] (32 nodes/group/chunk),
  DVE reduce over k=17 (strided 4D view) -> oblk[128, 8, 512],
  then 8 per-group DMAs write oblk back to out[C, N] row-major.
"""

import os

import numpy as np

import concourse.bacc as bacc
import concourse.bass as bass
import concourse.mybir as mybir
from concourse.bass_utils import run_bass_kernel_spmd

B, C, N, K = 16, 128, 4096, 16
N_CORES = 8
S = B // N_CORES
D = 8                      # channels per partition block
NG = 8                     # gpsimd groups
NODES_PER_GROUP = N // NG  # 512
CHUNKS = 16
KS = K + 1                            # 16 neighbors + self
NODES_PER_CHUNK = NODES_PER_GROUP // CHUNKS  # 32 nodes per group per chunk
NI = NODES_PER_CHUNK * KS            # 544 idx per gather
PIPELINED = os.environ.get("PIPELINE", "1") == "1"

_NC_CACHE = {}


def _build_program():
    nc = bacc.Bacc(None, target_bir_lowering=False)

    ncols = N // NG * KS // 16  # 544 idx columns per sample
    xe_d = nc.dram_tensor("xe", [S, C, N * D], mybir.dt.float32, kind="ExternalInput")
    idx_d = nc.dram_tensor("idx", [S, C, ncols], mybir.dt.int16,
                           kind="ExternalInput")
    out_d = nc.dram_tensor("out", [S, C, N], mybir.dt.float32, kind="ExternalOutput")

    with (
        nc.Block() as block,
        nc.semaphore("dsem") as dsem,   # input DMAs
        nc.semaphore("gsem") as gsem,   # gather chunks done
        nc.semaphore("vsem") as vsem,   # reduces done
        nc.semaphore("msem") as msem,   # per-sample maxes done
        nc.semaphore("osem") as osem,   # out DMAs done
        nc.sbuf_tensor("tbl", [C, N * D], mybir.dt.float32) as tbl,          # 128KB/p
        nc.sbuf_tensor("gt0", [C, NI * D], mybir.dt.float32) as gt0,         # 16KB/p
        nc.sbuf_tensor("gt1", [C, NI * D], mybir.dt.float32) as gt1,         # 16KB/p
        nc.sbuf_tensor("oblk", [C, D * N // NG], mybir.dt.float32) as oblk,  # 16KB/p
        nc.sbuf_tensor("idxt", [C, S * (N // NG) * KS // 16], mybir.dt.int16) as idxt,
        nc.sbuf_tensor("msem_probe", [1, 4], mybir.dt.float32) as msem_probe,
    ):
        gts = [gt0, gt1]

        @block.sync
        def _(sy: bass.BassEngine):
            # all idx up front (small)
            for s in range(S):
                sy.dma_start(out=idxt[:, s * ncols:(s + 1) * ncols],
                             in_=idx_d[s]).then_inc(dsem, 16)
            for s in range(S):
                if s > 0:
                    # table buffer reused: only the gathers read tbl, so it can
                    # be overwritten as soon as sample s-1's gathers retire
                    sy.wait_ge(gsem, CHUNKS * s)
                sy.dma_start(out=tbl[:], in_=xe_d[s]).then_inc(dsem, 16)

        @block.gpsimd
        def _(g: bass.BassGpSimd):
            for s in range(S):
                g.wait_ge(dsem, 16 * S + 16 * (s + 1))  # all idx + table s
                for c in range(CHUNKS):
                    ci = s * CHUNKS + c
                    if ci >= 2:
                        back = 1 if PIPELINED else 0
                        g.wait_ge(vsem, ci - back)
                    col0 = s * ncols + c * (NI // 16)
                    g.ap_gather(
                        out_ap=gts[ci % 2][:],
                        in_ap=tbl[:],
                        idxs_ap=idxt[:, col0:col0 + NI // 16],
                        channels=C, num_elems=N, d=D, num_idxs=NI,
                    ).then_inc(gsem, 1)

        @block.vector
        def _(v: bass.BassVectorEngine):
            for s in range(S):
                for c in range(CHUNKS):
                    ci = s * CHUNKS + c
                    v.wait_ge(gsem, ci + 1)
                    if s > 0 and c == 0:
                        v.wait_ge(osem, 256 * s)  # oblk drained (2x8 DMAs x16)
                    gt = gts[ci % 2]
                    gin = gt[:].rearrange("p (n k j) -> p j n k", k=KS, j=D)
                    oout = oblk[:].rearrange("p (j n) -> p j n", j=D)[
                        :, :, c * NODES_PER_CHUNK:(c + 1) * NODES_PER_CHUNK]
                    v.tensor_reduce(out=oout, in_=gin,
                                    axis=mybir.AxisListType.X,
                                    op=mybir.AluOpType.max).then_inc(vsem, 1)
                    if c == CHUNKS // 2 - 1:
                        # first half of oblk final -> early out-DMAs
                        v.wait_ge(vsem, s * CHUNKS + CHUNKS // 2)
                        v.memset(msem_probe[:1, :1], 0).then_inc(msem, 1)
                # all reduces done -> sample complete (self folded into gather)
                v.wait_ge(vsem, (s + 1) * CHUNKS)
                v.memset(msem_probe[:1, :1], 0).then_inc(msem, 1)

        @block.scalar
        def _(sc: bass.BassEngine):
            half = NODES_PER_GROUP // 2
            for s in range(S):
                for h in range(2):
                    sc.wait_ge(msem, 2 * s + h + 1)
                    for gg in range(NG):
                        src = oblk[gg * 16:(gg + 1) * 16].rearrange(
                            "p (j n) -> p j n", j=D)[:, :, h * half:(h + 1) * half]
                        dst = bass.AP(
                            out_d,
                            s * C * N + gg * NODES_PER_GROUP + h * half,
                            [[D * N, 16], [N, D], [1, half]],
                        )
                        sc.dma_start(out=dst, in_=src).then_inc(osem, 16)

    nc.compile()
    return nc


def _prep_sample(x_s: np.ndarray, nidx_s: np.ndarray):
    """x_s [C, N] f32, nidx_s [N, K] int -> (xe [C, N*D] f32, idx [C, N*K/16] i16)."""
    xq = x_s.reshape(16, D, N).transpose(0, 2, 1)          # [q, n, j]
    xe = np.broadcast_to(xq[None], (NG, 16, N, D)).reshape(C, N * D)
    idx16 = np.ascontiguousarray(nidx_s, dtype=np.int16)   # [N, K]
    self_col = np.arange(N, dtype=np.int16)[:, None]       # [N, 1]
    idx17 = np.concatenate([idx16, self_col], axis=1)      # [N, 17]
    blocks = []
    for g in range(NG):
        flat = idx17[g * NODES_PER_GROUP:(g + 1) * NODES_PER_GROUP].reshape(-1)
        blocks.append(flat.reshape(-1, 16).T)              # [16, N*KS/16/NG]
    idx = np.concatenate(blocks, axis=0)                   # [128, 544]
    return np.ascontiguousarray(xe), np.ascontiguousarray(idx)


def _run(x: np.ndarray, neighbor_idx: np.ndarray, **spmd_kwargs):
    x = np.asarray(x, dtype=np.float32)
    neighbor_idx = np.asarray(neighbor_idx)

    if "nc" not in _NC_CACHE:
        _NC_CACHE["nc"] = _build_program()
    nc = _NC_CACHE["nc"]

    in_maps = []
    for core in range(N_CORES):
        lo = core * S
        xes, idxs = [], []
        for s in range(S):
            xe, idx = _prep_sample(x[lo + s], neighbor_idx[lo + s])
            xes.append(xe)
            idxs.append(idx)
        in_maps.append({
            "xe": np.stack(xes, axis=0),
            "idx": np.stack(idxs, axis=0),
        })

    res = run_bass_kernel_spmd(nc, in_maps, core_ids=list(range(N_CORES)),
                               **spmd_kwargs)
    out = np.concatenate([res.results[core]["out"] for core in range(N_CORES)],
                         axis=0)
    return out.astype(np.float32), res


def kernel(x: np.ndarray, neighbor_idx: np.ndarray) -> np.ndarray:
    return _run(x, neighbor_idx)[0]


if __name__ == "__main__":
    rng = np.random.default_rng(0)
    xt = rng.standard_normal((B, C, N)).astype(np.float32)
    it = rng.integers(0, N, size=(B, N, K)).astype(np.int64)
    got = kernel(xt, it)
    ref = np.maximum(
        np.max(xt[np.arange(B)[:, None, None], :, it], axis=2).transpose(0, 2, 1),
        xt,
    )
    print("abs err:", np.abs(got - ref).max())



# revision 5
# speedup vs baseline: 1.2051x; 1.2051x over previous
"""GNN neighbor-max kernel — bf16 ap_gather + bf16 TT-max tree.

Per core: 2 samples (batch-parallel across 8 NeuronCores). Per sample:
  table xe[16g+q, n*8+j] = bf16(x[8q+j, n]) (one full copy per GPSIMD group,
  64KB/partition) so ONE gather index fetches all 128 channels of a node.
  Group g owns nodes [g*512, (g+1)*512); chunks of 64 nodes; idx packs 17
  slots per node (16 neighbors + self): 1088 idx per gather.
  8 chunks x 2 samples: ap_gather -> gt [128, 1088*8] bf16, then a DVE
  tensor_tensor max tree over k (j=8 innermost packed -> 2x_1p perf mode):
  8v8 -> 4v4 -> 2v2 -> 1v1 -> max vs self slot -> oblk bf16, DMA'd out as
  [S, C, N] bf16 (host casts back to f32).
"""

import numpy as np

import concourse.bacc as bacc
import concourse.bass as bass
import concourse.mybir as mybir
from concourse.bass_utils import run_bass_kernel_spmd

B, C, N, K = 16, 128, 4096, 16
N_CORES = 8
S = B // N_CORES
D = 8                       # channels per partition block
NG = 8                      # gpsimd groups (Q7 cores)
NODES_PER_GROUP = N // NG   # 512
NC = 64                     # nodes per group per chunk
CHUNKS = NODES_PER_GROUP // NC  # 8
KS = K + 1                  # 16 neighbors + self
NI = NC * KS                # 1088 idx per gather
ICOL = NI // 16             # 68 idx columns per chunk per partition

BF16 = mybir.dt.bfloat16
NPBF16 = mybir.dt.np(BF16)

_NC_CACHE = {}


def _build_program():
    nc = bacc.Bacc(None, target_bir_lowering=False)

    xe_d = nc.dram_tensor("xe", [S, C, N * D], BF16, kind="ExternalInput")
    idx_d = nc.dram_tensor("idx", [S, C, CHUNKS * ICOL], mybir.dt.int16,
                           kind="ExternalInput")
    out_d = nc.dram_tensor("out", [S, C, N], BF16, kind="ExternalOutput")

    with (
        nc.Block() as block,
        nc.semaphore("isem") as isem,    # idx DMAs
        nc.semaphore("t0sem") as t0sem,  # table 0 DMA
        nc.semaphore("t1sem") as t1sem,  # table 1 DMA
        nc.semaphore("gsem") as gsem,    # gather chunks done
        nc.semaphore("vsem") as vsem,    # chunk reduces done
        nc.semaphore("osem") as osem,    # out DMAs done
        nc.sbuf_tensor("tbl0", [C, N * D], BF16) as tbl0,        # 64KB/p
        nc.sbuf_tensor("tbl1", [C, N * D], BF16) as tbl1,        # 64KB/p
        nc.sbuf_tensor("gt0", [C, NI * D], BF16) as gt0,         # 17KB/p
        nc.sbuf_tensor("gt1", [C, NI * D], BF16) as gt1,         # 17KB/p
        nc.sbuf_tensor("t1b", [C, NC * 8 * D], BF16) as t1b,     # 8KB/p
        nc.sbuf_tensor("t2b", [C, NC * 4 * D], BF16) as t2b,     # 4KB/p
        nc.sbuf_tensor("t3b", [C, NC * 2 * D], BF16) as t3b,     # 2KB/p
        nc.sbuf_tensor("t4b", [C, NC * D], BF16) as t4b,         # 1KB/p
        nc.sbuf_tensor("ob0", [C, NODES_PER_GROUP * D], BF16) as ob0,  # 8KB/p
        nc.sbuf_tensor("ob1", [C, NODES_PER_GROUP * D], BF16) as ob1,  # 8KB/p
        nc.sbuf_tensor("idxt", [C, S * CHUNKS * ICOL], mybir.dt.int16) as idxt,
    ):
        tbls = [tbl0, tbl1]
        gts = [gt0, gt1]
        obs = [ob0, ob1]

        @block.sync
        def _(sy: bass.BassEngine):
            for s in range(S):
                sy.dma_start(
                    out=idxt[:, s * CHUNKS * ICOL:(s + 1) * CHUNKS * ICOL],
                    in_=idx_d[s]).then_inc(isem, 16)
            sy.dma_start(out=tbl0[:], in_=xe_d[0]).then_inc(t0sem, 16)
            sy.dma_start(out=tbl1[:], in_=xe_d[1]).then_inc(t1sem, 16)

        @block.gpsimd
        def _(g: bass.BassGpSimd):
            for ci in range(S * CHUNKS):
                s, c = divmod(ci, CHUNKS)
                if ci == 0:
                    g.wait_ge(isem, 16 * S)
                    g.wait_ge(t0sem, 16)
                if ci == CHUNKS:
                    g.wait_ge(t1sem, 16)
                if ci >= 2:
                    g.wait_ge(vsem, ci - 1)  # gt buffer free
                col0 = (s * CHUNKS + c) * ICOL
                g.ap_gather(
                    out_ap=gts[ci % 2][:],
                    in_ap=tbls[s][:],
                    idxs_ap=idxt[:, col0:col0 + ICOL],
                    channels=C, num_elems=N, d=D, num_idxs=NI,
                ).then_inc(gsem, 1)

        @block.vector
        def _(v: bass.BassVectorEngine):
            mx = mybir.AluOpType.max
            for ci in range(S * CHUNKS):
                s, c = divmod(ci, CHUNKS)
                v.wait_ge(gsem, ci + 1)
                g4 = gts[ci % 2][:].rearrange("p (n k j) -> p n k j", k=KS, j=D)
                t1v = t1b[:].rearrange("p (n k j) -> p n k j", k=8, j=D)
                t2v = t2b[:].rearrange("p (n k j) -> p n k j", k=4, j=D)
                t3v = t3b[:].rearrange("p (n k j) -> p n k j", k=2, j=D)
                t4v = t4b[:].rearrange("p (n o j) -> p n o j", o=1, j=D)
                # ob layout is (j n) so the out-DMA's last dim is contiguous
                ov = obs[s][:].rearrange("p (j o n) -> p n o j", j=D, o=1)[
                    :, c * NC:(c + 1) * NC, :, :]
                v.tensor_tensor(t1v, g4[:, :, 0:8, :], g4[:, :, 8:16, :], mx)
                v.tensor_tensor(t2v, t1v[:, :, 0:4, :], t1v[:, :, 4:8, :], mx)
                v.tensor_tensor(t3v, t2v[:, :, 0:2, :], t2v[:, :, 2:4, :], mx)
                v.tensor_tensor(t4v, t3v[:, :, 0:1, :], t3v[:, :, 1:2, :], mx)
                v.tensor_tensor(ov, t4v, g4[:, :, 16:17, :],
                                mx).then_inc(vsem, 1)

        @block.scalar
        def _(sc: bass.BassEngine):
            half = NODES_PER_GROUP // 2
            hc = CHUNKS // 2
            for s in range(S):
                for h in range(2):
                    sc.wait_ge(vsem, s * CHUNKS + (h + 1) * hc)
                    for gg in range(NG):
                        src = obs[s][gg * 16:(gg + 1) * 16].rearrange(
                            "p (j n) -> p j n", j=D)[:, :, h * half:(h + 1) * half]
                        dst = bass.AP(
                            out_d,
                            s * C * N + gg * NODES_PER_GROUP + h * half,
                            [[D * N, 16], [N, D], [1, half]],
                        )
                        sc.dma_start(out=dst, in_=src).then_inc(osem, 16)

    nc.compile()
    return nc


def _prep_sample(x_s: np.ndarray, nidx_s: np.ndarray):
    """x_s [C,N] f32, nidx_s [N,K] int -> (xe [C,N*8] bf16, idx [C,544] i16)."""
    xq = x_s.reshape(16, D, N).transpose(0, 2, 1)          # [q, n, j]
    xe = np.broadcast_to(xq[None], (NG, 16, N, D)).reshape(C, N * D)
    idx16 = np.ascontiguousarray(nidx_s, dtype=np.int16)   # [N, K]
    self_col = np.arange(N, dtype=np.int16)[:, None]       # [N, 1]
    idx17 = np.concatenate([idx16, self_col], axis=1)      # [N, 17]
    # per group g, per chunk c: 64 nodes x 17 slots wrapped into 16 partitions
    i4 = idx17.reshape(NG, CHUNKS, NC * KS)                # [g, c, 1088]
    i4 = i4.reshape(NG, CHUNKS, NI // 16, 16)              # [g, c, 68, 16]
    i4 = i4.transpose(0, 3, 1, 2)                          # [g, 16, c, 68]
    idx = i4.reshape(C, CHUNKS * ICOL)
    return np.ascontiguousarray(xe.astype(NPBF16)), np.ascontiguousarray(idx)


def _run(x: np.ndarray, neighbor_idx: np.ndarray, **spmd_kwargs):
    x = np.asarray(x, dtype=np.float32)
    neighbor_idx = np.asarray(neighbor_idx)

    if "nc" not in _NC_CACHE:
        _NC_CACHE["nc"] = _build_program()
    nc = _NC_CACHE["nc"]

    in_maps = []
    for core in range(N_CORES):
        lo = core * S
        xes, idxs = [], []
        for s in range(S):
            xe, idx = _prep_sample(x[lo + s], neighbor_idx[lo + s])
            xes.append(xe)
            idxs.append(idx)
        in_maps.append({
            "xe": np.stack(xes, axis=0),
            "idx": np.stack(idxs, axis=0),
        })

    res = run_bass_kernel_spmd(nc, in_maps, core_ids=list(range(N_CORES)),
                               **spmd_kwargs)
    out = np.concatenate(
        [res.results[core]["out"].astype(np.float32) for core in range(N_CORES)],
        axis=0)
    return out, res


def kernel(x: np.ndarray, neighbor_idx: np.ndarray) -> np.ndarray:
    return _run(x, neighbor_idx)[0]


if __name__ == "__main__":
    rng = np.random.default_rng(0)
    xt = rng.standard_normal((B, C, N)).astype(np.float32)
    it = rng.integers(0, N, size=(B, N, K)).astype(np.int64)
    got = kernel(xt, it)
    ref = np.maximum(
        np.max(xt[np.arange(B)[:, None, None], :, it], axis=2).transpose(0, 2, 1),
        xt,
    )
    print("abs err:", np.abs(got - ref).max())


# revision 13
# speedup vs baseline: 1.2799x; 1.0621x over previous
"""GNN neighbor-max kernel — bf16 ap_gather + bf16 TT-max tree (v2b: self-slot folded via table).

Per core: 2 samples, batch-parallel across the 8 NeuronCores. Per sample:
  table xe[16g+q, n*8+j] = bf16(x[8q+j, n]) so ONE gather index fetches all
  128 channels of a node. Group g owns nodes [g*512, (g+1)*512); chunks of
  64 nodes; 17 idx per node (16 neighbors + self): 1088 idx per ap_gather.
  DVE reduces each chunk with a bf16 tensor_tensor max tree (j=8 innermost
  packed -> 2x_1p): 8v8 -> 4v4 -> 2v2 -> 1v1 -> max vs self slot -> ob
  (j n layout) -> out [S, C, N] bf16 (host casts back to f32).
"""

import numpy as np

import concourse.bacc as bacc
import concourse.bass as bass
import concourse.mybir as mybir
from concourse.bass_utils import run_bass_kernel_spmd

B, C, N, K = 16, 128, 4096, 16
N_CORES = 8
S = B // N_CORES
D = 8                       # channels per partition block
NG = 8                      # gpsimd groups
NODES_PER_GROUP = N // NG   # 512
NC = 64                     # nodes per group per chunk
CHUNKS = NODES_PER_GROUP // NC  # 8
KS = K                      # 16 neighbors (self folded via tbl)
NI = NC * KS                # 1024 idx per gather
ICOL = NI // 16             # 68

BF16 = mybir.dt.bfloat16
NPBF16 = mybir.dt.np(BF16)

_NC_CACHE = {}


def _build_program():
    nc = bacc.Bacc(None, target_bir_lowering=False)

    xe_d = nc.dram_tensor("xe", [S, C, N * D], BF16, kind="ExternalInput")
    idx_d = nc.dram_tensor("idx", [S, C, CHUNKS * ICOL], mybir.dt.int16,
                           kind="ExternalInput")
    out_d = nc.dram_tensor("out", [S, C, N], BF16, kind="ExternalOutput")

    from contextlib import ExitStack

    with ExitStack() as es:
        block = es.enter_context(nc.Block())
        sem = lambda name: es.enter_context(nc.semaphore(name))  # noqa: E731
        buf = lambda name, shape, dt: es.enter_context(  # noqa: E731
            nc.sbuf_tensor(name, shape, dt))
        isem = sem("isem")
        t0sem = sem("t0sem")
        t1sem = sem("t1sem")
        gsem = sem("gsem")
        vsem = sem("vsem")
        osem = sem("osem")
        tbl0 = buf("tbl0", [C, N * D], BF16)
        tbl1 = buf("tbl1", [C, N * D], BF16)
        gt0 = buf("gt0", [C, NI * D], BF16)
        gt1 = buf("gt1", [C, NI * D], BF16)
        t1b = buf("t1b", [C, NC * 8 * D], BF16)
        t2b = buf("t2b", [C, NC * 4 * D], BF16)
        t3b = buf("t3b", [C, NC * 2 * D], BF16)
        t4b = buf("t4b", [C, NC * D], BF16)
        ob0 = buf("ob0", [C, NODES_PER_GROUP * D], BF16)
        ob1 = buf("ob1", [C, NODES_PER_GROUP * D], BF16)
        idxt = buf("idxt", [C, S * CHUNKS * ICOL], mybir.dt.int16)

        tbls = [tbl0, tbl1]
        gts = [gt0, gt1]
        obs = [ob0, ob1]

        @block.sync
        def _(sy: bass.BassEngine):
            for s in range(S):
                sy.dma_start(
                    out=idxt[:, s * CHUNKS * ICOL:(s + 1) * CHUNKS * ICOL],
                    in_=idx_d[s]).then_inc(isem, 16)
            sy.dma_start(out=tbl0[:], in_=xe_d[0]).then_inc(t0sem, 16)
            sy.dma_start(out=tbl1[:], in_=xe_d[1]).then_inc(t1sem, 16)

        @block.gpsimd
        def _(g: bass.BassGpSimd):
            for ci in range(S * CHUNKS):
                s, c = divmod(ci, CHUNKS)
                if ci == 0:
                    g.wait_ge(isem, 16 * S)
                    g.wait_ge(t0sem, 16)
                if ci == CHUNKS:
                    g.wait_ge(t1sem, 16)
                if ci >= 2:
                    g.wait_ge(vsem, ci - 1)
                col0 = (s * CHUNKS + c) * ICOL
                g.ap_gather(
                    out_ap=gts[ci % 2][:],
                    in_ap=tbls[s][:],
                    idxs_ap=idxt[:, col0:col0 + ICOL],
                    channels=C, num_elems=N, d=D, num_idxs=NI,
                ).then_inc(gsem, 1)

        @block.vector
        def _(v: bass.BassVectorEngine):
            mx = mybir.AluOpType.max
            for ci in range(S * CHUNKS):
                s, c = divmod(ci, CHUNKS)
                v.wait_ge(gsem, ci + 1)
                g4 = gts[ci % 2][:].rearrange("p (n k j) -> p n k j", k=KS, j=D)
                t1v = t1b[:].rearrange("p (n k j) -> p n k j", k=8, j=D)
                t2v = t2b[:].rearrange("p (n k j) -> p n k j", k=4, j=D)
                t3v = t3b[:].rearrange("p (n k j) -> p n k j", k=2, j=D)
                t4v = t4b[:].rearrange("p (n o j) -> p n o j", o=1, j=D)
                v.tensor_tensor(t1v, g4[:, :, 0:8, :], g4[:, :, 8:16, :], mx)
                v.tensor_tensor(t2v, t1v[:, :, 0:4, :], t1v[:, :, 4:8, :], mx)
                v.tensor_tensor(t3v, t2v[:, :, 0:2, :], t2v[:, :, 2:4, :], mx)
                v.tensor_tensor(t4v, t3v[:, :, 0:1, :], t3v[:, :, 1:2, :], mx)
                # table is rotated per group: group g's own nodes sit at
                # positions [0, 512), so the self-slice offset is uniform
                selfv = tbls[s][:, c * NC * D:(c * NC + NC) * D].rearrange(
                    "p (n o j) -> p n o j", o=1, j=D)
                # ob layout is (j n) so the out-DMA's last dim is contiguous
                ov = obs[s][:].rearrange("p (j o n) -> p n o j", j=D, o=1)[
                    :, c * NC:(c + 1) * NC, :, :]
                v.tensor_tensor(ov, t4v, selfv, mx).then_inc(vsem, 1)

        @block.scalar
        def _(sc: bass.BassEngine):
            half = NODES_PER_GROUP // 2
            hc = CHUNKS // 2
            for s in range(S):
                for h in range(2):
                    sc.wait_ge(vsem, s * CHUNKS + (h + 1) * hc)
                    for gg in range(NG):
                        src = obs[s][gg * 16:(gg + 1) * 16].rearrange(
                            "p (j n) -> p j n", j=D)[:, :, h * half:(h + 1) * half]
                        dst = bass.AP(
                            out_d,
                            s * C * N + gg * NODES_PER_GROUP + h * half,
                            [[D * N, 16], [N, D], [1, half]],
                        )
                        sc.dma_start(out=dst, in_=src).then_inc(osem, 16)

    nc.compile()
    return nc


def _prep_sample(x_s: np.ndarray, nidx_s: np.ndarray):
    xq = x_s.reshape(16, D, N).transpose(0, 2, 1)      # [q, n, j]
    # per-group rotated table: group g position m holds node (m + g*512) % N
    xe = np.stack([np.roll(xq, -g * NODES_PER_GROUP, axis=1)
                   for g in range(NG)], axis=0).reshape(C, N * D)
    idx16 = np.ascontiguousarray(nidx_s, dtype=np.int16)   # [N, K]
    # remap idx to each group's rotated table coordinates
    goff = (np.arange(NG, dtype=np.int32)[:, None, None]
            * NODES_PER_GROUP)                              # [g, 1, 1]
    idx_g = (idx16.reshape(NG, NODES_PER_GROUP, KS).astype(np.int32)
             - goff) % N
    i4 = idx_g.astype(np.int16).reshape(NG, CHUNKS, NC * KS)
    i4 = i4.reshape(NG, CHUNKS, NI // 16, 16)
    i4 = i4.transpose(0, 3, 1, 2)
    idx = i4.reshape(C, CHUNKS * ICOL)
    return np.ascontiguousarray(xe.astype(NPBF16)), np.ascontiguousarray(idx)


def _run(x: np.ndarray, neighbor_idx: np.ndarray, **spmd_kwargs):
    x = np.asarray(x, dtype=np.float32)
    neighbor_idx = np.asarray(neighbor_idx)

    if "nc" not in _NC_CACHE:
        _NC_CACHE["nc"] = _build_program()
    nc = _NC_CACHE["nc"]

    in_maps = []
    for core in range(N_CORES):
        lo = core * S
        xes, idxs = [], []
        for s in range(S):
            xe, idx = _prep_sample(x[lo + s], neighbor_idx[lo + s])
            xes.append(xe)
            idxs.append(idx)
        in_maps.append({
            "xe": np.stack(xes, axis=0),
            "idx": np.stack(idxs, axis=0),
        })

    res = run_bass_kernel_spmd(nc, in_maps, core_ids=list(range(N_CORES)),
                               **spmd_kwargs)
    out = np.concatenate(
        [res.results[core]["out"].astype(np.float32) for core in range(N_CORES)],
        axis=0)
    return out, res


def kernel(x: np.ndarray, neighbor_idx: np.ndarray) -> np.ndarray:
    return _run(x, neighbor_idx)[0]


if __name__ == "__main__":
    rng = np.random.default_rng(0)
    xt = rng.standard_normal((B, C, N)).astype(np.float32)
    it = rng.integers(0, N, size=(B, N, K)).astype(np.int64)
    got = kernel(xt, it)
    ref = np.maximum(
        np.max(xt[np.arange(B)[:, None, None], :, it], axis=2).transpose(0, 2, 1),
        xt,
    )
    print("abs err:", np.abs(got - ref).max())


# revision 14
# speedup vs baseline: 1.2832x; 1.0026x over previous
"""GNN neighbor-max kernel — bf16 ap_gather + bf16 TT-max tree (v2b: self-slot folded via table).

Per core: 2 samples, batch-parallel across the 8 NeuronCores. Per sample:
  table xe[16g+q, n*8+j] = bf16(x[8q+j, n]) so ONE gather index fetches all
  128 channels of a node. Group g owns nodes [g*512, (g+1)*512); chunks of
  64 nodes; 17 idx per node (16 neighbors + self): 1088 idx per ap_gather.
  DVE reduces each chunk with a bf16 tensor_tensor max tree (j=8 innermost
  packed -> 2x_1p): 8v8 -> 4v4 -> 2v2 -> 1v1 -> max vs self slot -> ob
  (j n layout) -> out [S, C, N] bf16 (host casts back to f32).
"""

import numpy as np

import concourse.bacc as bacc
import concourse.bass as bass
import concourse.mybir as mybir
from concourse.bass_utils import run_bass_kernel_spmd

B, C, N, K = 16, 128, 4096, 16
N_CORES = 8
S = B // N_CORES
D = 8                       # channels per partition block
NG = 8                      # gpsimd groups
NODES_PER_GROUP = N // NG   # 512
NC = 64                     # nodes per group per chunk
CHUNKS = NODES_PER_GROUP // NC  # 8
KS = K                      # 16 neighbors (self folded via tbl)
NI = NC * KS                # 1024 idx per gather
ICOL = NI // 16             # 68

BF16 = mybir.dt.bfloat16
NPBF16 = mybir.dt.np(BF16)

_NC_CACHE = {}


def _build_program():
    nc = bacc.Bacc(None, target_bir_lowering=False)

    xe_d = nc.dram_tensor("xe", [S, C, N * D], BF16, kind="ExternalInput")
    idx_d = nc.dram_tensor("idx", [S, C, CHUNKS * ICOL], mybir.dt.int16,
                           kind="ExternalInput")
    out_d = nc.dram_tensor("out", [S, C, N], BF16, kind="ExternalOutput")

    from contextlib import ExitStack

    with ExitStack() as es:
        block = es.enter_context(nc.Block())
        sem = lambda name: es.enter_context(nc.semaphore(name))  # noqa: E731
        buf = lambda name, shape, dt: es.enter_context(  # noqa: E731
            nc.sbuf_tensor(name, shape, dt))
        isem = sem("isem")
        t0sem = sem("t0sem")
        t1sem = sem("t1sem")
        gsem = sem("gsem")
        vsem = sem("vsem")
        osem = sem("osem")
        tbl0 = buf("tbl0", [C, N * D], BF16)
        tbl1 = buf("tbl1", [C, N * D], BF16)
        gt0 = buf("gt0", [C, NI * D], BF16)
        gt1 = buf("gt1", [C, NI * D], BF16)
        t1b = buf("t1b", [C, NC * 8 * D], BF16)
        t2b = buf("t2b", [C, NC * 4 * D], BF16)
        t3b = buf("t3b", [C, NC * 2 * D], BF16)
        t4b = buf("t4b", [C, NC * D], BF16)
        ob0 = buf("ob0", [C, NODES_PER_GROUP * D], BF16)
        ob1 = buf("ob1", [C, NODES_PER_GROUP * D], BF16)
        idxt = buf("idxt", [C, S * CHUNKS * ICOL], mybir.dt.int16)
        dscr = buf("dscr", [C, 16], mybir.dt.int16)
        dtbl = buf("dtbl", [C, 32], BF16)

        tbls = [tbl0, tbl1]
        gts = [gt0, gt1]
        obs = [ob0, ob1]

        @block.sync
        def _(sy: bass.BassEngine):
            for s in range(S):
                sy.dma_start(
                    out=idxt[:, s * CHUNKS * ICOL:(s + 1) * CHUNKS * ICOL],
                    in_=idx_d[s]).then_inc(isem, 16)
            sy.dma_start(out=tbl0[:], in_=xe_d[0]).then_inc(t0sem, 16)
            sy.dma_start(out=tbl1[:], in_=xe_d[1]).then_inc(t1sem, 16)

        @block.gpsimd
        def _(g: bass.BassGpSimd):
            # warm the ap_gather ucode library while the tables stream in
            g.memset(dscr[:], 0)
            g.memset(dtbl[:], 0.0)
            g.ap_gather(out_ap=gt0[:, 0:128], in_ap=dtbl[:],
                        idxs_ap=dscr[:, 0:1], channels=C, num_elems=4,
                        d=D, num_idxs=16)
            for ci in range(S * CHUNKS):
                s, c = divmod(ci, CHUNKS)
                if ci == 0:
                    g.wait_ge(isem, 16 * S)
                    g.wait_ge(t0sem, 16)
                if ci == CHUNKS:
                    g.wait_ge(t1sem, 16)
                if ci >= 2:
                    g.wait_ge(vsem, ci - 1)
                col0 = (s * CHUNKS + c) * ICOL
                g.ap_gather(
                    out_ap=gts[ci % 2][:],
                    in_ap=tbls[s][:],
                    idxs_ap=idxt[:, col0:col0 + ICOL],
                    channels=C, num_elems=N, d=D, num_idxs=NI,
                ).then_inc(gsem, 1)

        @block.vector
        def _(v: bass.BassVectorEngine):
            mx = mybir.AluOpType.max
            for ci in range(S * CHUNKS):
                s, c = divmod(ci, CHUNKS)
                v.wait_ge(gsem, ci + 1)
                g4 = gts[ci % 2][:].rearrange("p (n k j) -> p n k j", k=KS, j=D)
                t1v = t1b[:].rearrange("p (n k j) -> p n k j", k=8, j=D)
                t2v = t2b[:].rearrange("p (n k j) -> p n k j", k=4, j=D)
                t3v = t3b[:].rearrange("p (n k j) -> p n k j", k=2, j=D)
                t4v = t4b[:].rearrange("p (n o j) -> p n o j", o=1, j=D)
                v.tensor_tensor(t1v, g4[:, :, 0:8, :], g4[:, :, 8:16, :], mx)
                v.tensor_tensor(t2v, t1v[:, :, 0:4, :], t1v[:, :, 4:8, :], mx)
                v.tensor_tensor(t3v, t2v[:, :, 0:2, :], t2v[:, :, 2:4, :], mx)
                v.tensor_tensor(t4v, t3v[:, :, 0:1, :], t3v[:, :, 1:2, :], mx)
                # table is rotated per group: group g's own nodes sit at
                # positions [0, 512), so the self-slice offset is uniform
                selfv = tbls[s][:, c * NC * D:(c * NC + NC) * D].rearrange(
                    "p (n o j) -> p n o j", o=1, j=D)
                # ob layout is (j n) so the out-DMA's last dim is contiguous
                ov = obs[s][:].rearrange("p (j o n) -> p n o j", j=D, o=1)[
                    :, c * NC:(c + 1) * NC, :, :]
                v.tensor_tensor(ov, t4v, selfv, mx).then_inc(vsem, 1)

        @block.scalar
        def _(sc: bass.BassEngine):
            qn = NODES_PER_GROUP // 4
            qc = CHUNKS // 4
            for s in range(S):
                for h in range(4):
                    sc.wait_ge(vsem, s * CHUNKS + (h + 1) * qc)
                    for gg in range(NG):
                        src = obs[s][gg * 16:(gg + 1) * 16].rearrange(
                            "p (j n) -> p j n", j=D)[:, :, h * qn:(h + 1) * qn]
                        dst = bass.AP(
                            out_d,
                            s * C * N + gg * NODES_PER_GROUP + h * qn,
                            [[D * N, 16], [N, D], [1, qn]],
                        )
                        sc.dma_start(out=dst, in_=src).then_inc(osem, 16)

    nc.compile()
    return nc


def _prep_sample(x_s: np.ndarray, nidx_s: np.ndarray):
    xq = x_s.reshape(16, D, N).transpose(0, 2, 1)      # [q, n, j]
    # per-group rotated table: group g position m holds node (m + g*512) % N
    xe = np.stack([np.roll(xq, -g * NODES_PER_GROUP, axis=1)
                   for g in range(NG)], axis=0).reshape(C, N * D)
    idx16 = np.ascontiguousarray(nidx_s, dtype=np.int16)   # [N, K]
    # remap idx to each group's rotated table coordinates
    goff = (np.arange(NG, dtype=np.int32)[:, None, None]
            * NODES_PER_GROUP)                              # [g, 1, 1]
    idx_g = (idx16.reshape(NG, NODES_PER_GROUP, KS).astype(np.int32)
             - goff) % N
    i4 = idx_g.astype(np.int16).reshape(NG, CHUNKS, NC * KS)
    i4 = i4.reshape(NG, CHUNKS, NI // 16, 16)
    i4 = i4.transpose(0, 3, 1, 2)
    idx = i4.reshape(C, CHUNKS * ICOL)
    return np.ascontiguousarray(xe.astype(NPBF16)), np.ascontiguousarray(idx)


def _run(x: np.ndarray, neighbor_idx: np.ndarray, **spmd_kwargs):
    x = np.asarray(x, dtype=np.float32)
    neighbor_idx = np.asarray(neighbor_idx)

    if "nc" not in _NC_CACHE:
        _NC_CACHE["nc"] = _build_program()
    nc = _NC_CACHE["nc"]

    in_maps = []
    for core in range(N_CORES):
        lo = core * S
        xes, idxs = [], []
        for s in range(S):
            xe, idx = _prep_sample(x[lo + s], neighbor_idx[lo + s])
            xes.append(xe)
            idxs.append(idx)
        in_maps.append({
            "xe": np.stack(xes, axis=0),
            "idx": np.stack(idxs, axis=0),
        })

    res = run_bass_kernel_spmd(nc, in_maps, core_ids=list(range(N_CORES)),
                               **spmd_kwargs)
    out = np.concatenate(
        [res.results[core]["out"].astype(np.float32) for core in range(N_CORES)],
        axis=0)
    return out, res


def kernel(x: np.ndarray, neighbor_idx: np.ndarray) -> np.ndarray:
    return _run(x, neighbor_idx)[0]


if __name__ == "__main__":
    rng = np.random.default_rng(0)
    xt = rng.standard_normal((B, C, N)).astype(np.float32)
    it = rng.integers(0, N, size=(B, N, K)).astype(np.int64)
    got = kernel(xt, it)
    ref = np.maximum(
        np.max(xt[np.arange(B)[:, None, None], :, it], axis=2).transpose(0, 2, 1),
        xt,
    )
    print("abs err:", np.abs(got - ref).max())
